# revision 45
# baseline (speedup 1.0000x reference)
"""Sparse (graph-edge) multi-head attention block on 8 TRN2 NeuronCores.

Problem: nn_MultiHeadAttention_6966436954266
  B=2, N=20000, D=256, H=8, dh=32, E=160000 (8 out-edges per node, sorted by src)

  xn  = LN1(x); q,k,v = xn @ w{q,k,v}; per-edge w = exp(q_src.k_dst/sqrt(dh))
  attn = segment_sum(w*v_dst)/segment_sum(w); concat = xn + attn
  out = relu(LN2(concat) @ wo + bo) + concat

Sharding: nodes partitioned contiguously across 8 cores (2500 each). Each core
LN+projects its shard, bf16 K/V row tables are AllGathered (one 41MB Shared
buffer per core), then each core runs the edge stage for its own nodes' edges
(edge list grouped by src on host, padded to uniform R slots per node).

Edge-stage layout per 128-edge tile (16 nodes x 8 slots, SLOT-major:
edge position p = slot*16 + node so broadcast operands get step-1 inner
dims -> DVE 2x mode):
  - kv rows are [K_b0|K_b1|V_b0|V_b1] (V columns x-major); ONE row-wise
    dma_gather per 512 edges fetches the full 2KB row -- K and V for BOTH
    batches. SWDGE descriptor gen on the GpSimd Q7 (~4.8us/512 idxs) is
    the edge-stage bottleneck, so one gather instead of four.
  - q kept as bf16 ROWS; per-tile q-replication (x8 slots) via PE:
    qrep = R8.T @ q_tile (R8 [16,128] const one-hot) -> PSUM -> ACT copy
  - prod = K_rows * qrep on DVE (bf16 2x); logits = tensor_reduce over
    the 32-dim head groups -> [128(e), 16(b,h)] f32
  - exp on ACT; pad mask is a per-partition scalar mul
  - wv = v*w with free-dim broadcast (DVE 2x; V x-major so the inner
    iteration dim is the step-1 head axis)
  - segment sum via PE: lhsT = S (constant 0/1 [128,128]), rhs = wv
    [128,512] (one full PSUM bank) + a tiny denominator matmul [128,16]
  - AllGather runs in 2 phases (rows 0:1280, 1280:2500) so phase 1
    overlaps the second half of the LN/projection head
"""

import math
from dataclasses import dataclass

import numpy as np

import concourse.bass as bass
import concourse.bacc as bacc
import concourse.mybir as mybir
import concourse.tile as tile
from concourse.masks import make_identity

B, N, D, H, DH = 2, 20000, 256, 8, 32
NCORES = 8
EPS = 1e-3
P = 128
F32 = mybir.dt.float32
BF16 = mybir.dt.bfloat16
I16 = mybir.dt.int16
AF = mybir.ActivationFunctionType
ALU = mybir.AluOpType
SUBT = 4  # tiles per dma_gather (512-index SWDGE ring limit)


@dataclass(frozen=True)
class Cfg:
    n: int = N
    nloc: int = N // NCORES
    ncores: int = NCORES
    b: int = B
    r: int = 8          # edge slots per node (pow2, divides 128)
    gt: int = 16        # edge tiles per gather group
    mask_all: bool = False
    stages: int = 5      # debug: 2 = stop after collective, 4 = skip stage5
    apply_gb1: bool = False
    apply_gb2: bool = False
    apply_bqkv: bool = False
    apply_bo: bool = False

    @property
    def npt(self):
        return P // self.r

    @property
    def nt(self):
        return math.ceil(self.nloc / self.npt)

    @property
    def nloc_pad(self):
        return self.nt * self.npt

    @property
    def nt_pad(self):
        return _ceil_div(self.nt, SUBT) * SUBT

    @property
    def rt(self):
        return math.ceil(self.nloc / P)

    @property
    def ag_bounds(self):
        """AllGather phase row boundaries [0, ..., nloc]: small early
        phases pipeline behind the projection head; ncfw runs phases
        back-to-back so the tail phase hides under earlier transfers."""
        splits = [3, 10]  # in 128-row tiles
        bounds = [0] + [s * P for s in splits if s * P < self.nloc]
        return bounds + [self.nloc]

    @property
    def groups(self):
        return [(s, min(s + self.gt, self.nt)) for s in range(0, self.nt, self.gt)]


def _ceil_div(a, b):
    return (a + b - 1) // b


# ------------------------------------------------------------------- program
def build_program(cfg: Cfg) -> bass.Bass:
    nc = bacc.Bacc("TRN2", num_devices=cfg.ncores)
    d = D
    rt, nt = cfg.rt, cfg.nt
    nloc, b_ = cfg.nloc, cfg.b

    x_in = nc.dram_tensor("x", [b_ * nloc, d], F32, kind="ExternalInput")
    wqkv_in = nc.dram_tensor("wqkv", [d, 3 * d], BF16, kind="ExternalInput")
    wo_in = nc.dram_tensor("wo", [d, d], BF16, kind="ExternalInput")
    idx_in = nc.dram_tensor("idx", [P, cfg.nt_pad * (P // 16)], I16,
                        kind="ExternalInput")
    mask_in = nc.dram_tensor("mask", [P, nt], F32, kind="ExternalInput")
    sseg_in = nc.dram_tensor("sseg", [P, cfg.r * P], BF16, kind="ExternalInput")
    r8_in = nc.dram_tensor("r8", [P, 4 * P], BF16, kind="ExternalInput")
    gb_in = nc.dram_tensor("gb", [4, d], F32, kind="ExternalInput")
    brow_in = nc.dram_tensor("brow", [1, 4 * d], F32, kind="ExternalInput")
    out_ext = nc.dram_tensor("out", [b_ * nloc, d], F32, kind="ExternalOutput")

    kv_loc = nc.dram_tensor("kv_loc", [nloc, 4 * d], BF16)
    kv_full = nc.dram_tensor(
        "kv_full", [cfg.n, 4 * d], BF16,
        addr_space="Shared" if cfg.ncores > 4 else "Local")

    with tile.TileContext(nc) as tc:
        with tc.tile_pool(name="const", bufs=1) as cpool:
            ident_f = cpool.tile([P, P], F32)
            make_identity(nc, ident_f[:])
            ident_b = cpool.tile([P, P], BF16)
            make_identity(nc, ident_b[:])
            wqkv_sb = cpool.tile([P, 2, 3 * d], BF16)
            nc.sync.dma_start(
                out=wqkv_sb[:],
                in_=wqkv_in[:].rearrange("(ci p) q -> p ci q", p=P))
            wo_sb = cpool.tile([P, 2, d], BF16)
            nc.sync.dma_start(
                out=wo_sb[:], in_=wo_in[:].rearrange("(ci p) q -> p ci q", p=P))
            sseg_sb = cpool.tile([P, cfg.r, P], BF16)
            nc.sync.dma_start(out=sseg_sb[:], in_=sseg_in[:].rearrange("p (k m) -> p k m", k=cfg.r))
            r8_sb = cpool.tile([P, 4, P], BF16)
            nc.sync.dma_start(
                out=r8_sb[:], in_=r8_in[:].rearrange("n (t e) -> n t e", t=4))
            mask_sb = cpool.tile([P, nt], F32)
            nc.sync.dma_start(out=mask_sb[:], in_=mask_in[:])
            eps_sb = cpool.tile([P, 1], F32)
            nc.vector.memset(eps_sb[:], EPS)
            gb_sb = cpool.tile([1, 4, d], F32)
            if cfg.apply_gb1 or cfg.apply_gb2:
                nc.sync.dma_start(out=gb_sb[:],
                                  in_=gb_in[:].rearrange("g d -> 1 g d"))
            brow_sb = cpool.tile([1, 4 * d], F32)
            if cfg.apply_bqkv or cfg.apply_bo:
                nc.sync.dma_start(out=brow_sb[:], in_=brow_in[:])

            with tc.tile_pool(name="resident", bufs=1) as rpool:
                xn_sb = rpool.tile([P, b_ * rt, d], F32)
                xnt_sb = rpool.tile([P, 2, b_ * rt, P], BF16)
                q_sb = rpool.tile([P, rt, b_, d], BF16)
                _stage12(nc, tc, cfg, x_in, xn_sb, xnt_sb, ident_f, eps_sb,
                         gb_sb, wqkv_sb, brow_sb, kv_loc, kv_full)
                # Q rows are only needed by stage4; traced after the
                # collective triggers so PE/DVE overlap the AllGather.
                _stage2_q(nc, tc, cfg, xnt_sb, q_sb, wqkv_sb, brow_sb)
                tail = (wo_sb, ident_b, gb_sb, brow_sb, out_ext) \
                    if cfg.stages >= 5 else None
                if cfg.stages >= 4:
                    _stage4(nc, tc, cfg, idx_in, kv_full, q_sb, xn_sb,
                            sseg_sb, r8_sb, mask_sb, eps_sb, tail)
                if cfg.stages < 5:
                    _debug_out(nc, tc, cfg, xn_sb, kv_full, out_ext)
    nc.finalize()
    return nc


def _debug_out(nc, tc, cfg, xn_sb, kv_full, out_ext):
    """stages<5 debug tail: write concat rows (stage4 result) to out."""
    d, rt, nloc = D, cfg.rt, cfg.nloc
    with tc.tile_pool(name="dbg", bufs=2) as pool:
        for b in range(cfg.b):
            for irt in range(rt):
                bt = b * rt + irt
                rows = min(P, nloc - irt * P)
                ot = pool.tile([P, d], F32, tag="dot")
                nc.vector.tensor_copy(out=ot[:rows], in_=xn_sb[:rows, bt, :])
                nc.sync.dma_start(
                    out=out_ext[b * nloc + irt * P:
                                b * nloc + irt * P + rows, :],
                    in_=ot[:rows])


def _layer_norm_rs(nc, pool, src_ap, rows, eps_sb):
    """bn_stats -> mv [P,2] f32 with [:,0]=mean, [:,1]=1/sqrt(var+eps)."""
    stats = pool.tile([P, 6], F32, tag="ln_stats")
    nc.vector.bn_stats(out=stats[:rows], in_=src_ap)
    mv = pool.tile([P, 2], F32, tag="ln_mv")
    nc.vector.bn_aggr(out=mv[:rows], in_=stats[:rows])
    nc.scalar.activation(out=mv[:rows, 1:2], in_=mv[:rows, 1:2], func=AF.Sqrt,
                         bias=eps_sb[:rows], scale=1.0)
    nc.vector.reciprocal(out=mv[:rows, 1:2], in_=mv[:rows, 1:2])
    return mv


def _stage12(nc, tc, cfg, x_in, xn_sb, xnt_sb, ident_f, eps_sb, gb_sb,
             wqkv_sb, brow_sb, kv_loc, kv_full):
    """Fused LN1 + K/V projection head.

    One loop per irt tile-pair so the PE instruction stream interleaves
    transposes and projection matmuls (engine streams run near in-order:
    a separate stage2 loop made every kv matmul queue behind ALL
    transposes, delaying the first AllGather by ~70us).  LN stats are
    grouped per pair: one sqrt/recip ACT round trip per 2 tiles.  The
    AllGather runs in phases (cfg.ag_bounds), each issued as soon as its
    rows land.
    """
    d, rt, nloc = D, cfg.rt, cfg.nloc
    rg = [list(range(cfg.ncores))]
    bounds = cfg.ag_bounds
    row_off = 0  # kv_full row offset of the current phase
    with tc.tile_pool(name="s1", bufs=4) as pool, \
         tc.tile_pool(name="s1p", bufs=4, space="PSUM") as ppool, \
         tc.tile_pool(name="s2p", bufs=3, space="PSUM") as kpool:
        def emit_ln(irt, b):
            bt = b * rt + irt
            rows = min(P, nloc - irt * P)
            xt = pool.tile([P, d], F32, tag="xt")
            nc.sync.dma_start(
                out=xt[:rows],
                in_=x_in[b * nloc + irt * P: b * nloc + irt * P + rows, :])
            if rows < P:
                nc.vector.memset(xn_sb[:, bt, :], 0.0)
            mv = _layer_norm_rs(nc, pool, xt[:rows], rows, eps_sb)
            nc.vector.tensor_scalar(
                out=xn_sb[:rows, bt, :], in0=xt[:rows],
                scalar1=mv[:rows, 0:1], scalar2=mv[:rows, 1:2],
                op0=ALU.subtract, op1=ALU.mult)
            if cfg.apply_gb1:
                nc.vector.tensor_tensor(
                    out=xn_sb[:rows, bt, :], in0=xn_sb[:rows, bt, :],
                    in1=gb_sb[:, 0, :].partition_broadcast(rows),
                    op=ALU.mult)
                nc.vector.tensor_tensor(
                    out=xn_sb[:rows, bt, :], in0=xn_sb[:rows, bt, :],
                    in1=gb_sb[:, 1, :].partition_broadcast(rows),
                    op=ALU.add)

        def emit_tkv(irt, b):
            bt = b * rt + irt
            rows = min(P, nloc - irt * P)
            for ci in range(2):
                pt = ppool.tile([P, P], F32, tag="tr")
                nc.tensor.transpose(
                    out=pt[:], in_=xn_sb[:, bt, ci * P:(ci + 1) * P],
                    identity=ident_f[:])
                # split the PSUM->SBUF copies across ACT and DVE so
                # neither engine gates the head pipeline
                if ci == 0:
                    nc.scalar.copy(out=xnt_sb[:, ci, bt, :], in_=pt[:])
                else:
                    nc.vector.tensor_copy(out=xnt_sb[:, ci, bt, :],
                                          in_=pt[:])
            ps = kpool.tile([P, 2 * d], F32, tag="kv")
            for ci in range(2):
                nc.tensor.matmul(
                    out=ps[:], lhsT=xnt_sb[:, ci, bt, :],
                    rhs=wqkv_sb[:, ci, d:3 * d],
                    start=(ci == 0), stop=(ci == 1))
            kvb = pool.tile([P, 2 * d], BF16, tag="kvb")
            if cfg.apply_bqkv:
                nc.vector.tensor_tensor(
                    out=kvb[:rows], in0=ps[:rows],
                    in1=brow_sb[:, d:3 * d].partition_broadcast(rows),
                    op=ALU.add)
            else:
                nc.scalar.copy(out=kvb[:rows], in_=ps[:rows])
            # row layout [K0|K1|V0|V1]: K_b -> col d*b, V_b -> col 2d+d*b
            nc.sync.dma_start(
                out=kv_loc[irt * P: irt * P + rows]
                .rearrange("r (u c) -> r u c", c=d)[:, b::2, :],
                in_=kvb[:rows].rearrange("p (u c) -> p u c", c=d))

        # 2-stage software pipeline: emit tile k's LN before tile k-1's
        # transpose+projection so each in-order engine queue always has
        # independent work while the other engines run tile k-1's chain.
        prev = None
        for irt in range(rt):
            for b in range(cfg.b):
                emit_ln(irt, b)
                if prev is not None:
                    emit_tkv(*prev)
                prev = (irt, b)
            if irt == rt - 1:
                emit_tkv(*prev)
                prev = None
            done = min((irt + 1) * P, nloc) - (P if prev is not None else 0)
            while len(bounds) >= 2 and bounds[1] <= done:
                lo, hi = bounds[0], bounds[1]
                bounds = bounds[1:]
                nc.gpsimd.collective_compute(
                    "AllGather", ALU.bypass, replica_groups=rg,
                    ins=[kv_loc[lo:hi, :]],
                    outs=[kv_full[row_off: row_off
                                  + cfg.ncores * (hi - lo), :]])
                row_off += cfg.ncores * (hi - lo)


def _stage2_q(nc, tc, cfg, xnt_sb, q_sb, wqkv_sb, brow_sb):
    """Q projection kept as bf16 ROWS: q_sb[p, irt, b, :] = q[node, b]."""
    d, rt, nloc = D, cfg.rt, cfg.nloc
    with tc.tile_pool(name="s2q", bufs=3) as pool, \
         tc.tile_pool(name="s2qp", bufs=2, space="PSUM") as ppool:
        for irt in range(rt):
            for b in range(cfg.b):
                bt = b * rt + irt
                rows = min(P, nloc - irt * P)
                ps = ppool.tile([P, d], F32, tag="q")
                for ci in range(2):
                    nc.tensor.matmul(
                        out=ps[:], lhsT=xnt_sb[:, ci, bt, :],
                        rhs=wqkv_sb[:, ci, 0:d],
                        start=(ci == 0), stop=(ci == 1))
                if rows < P:
                    nc.vector.memset(q_sb[:, irt, b, :], 0.0)
                if cfg.apply_bqkv:
                    nc.vector.tensor_tensor(
                        out=q_sb[:rows, irt, b, :], in0=ps[:rows],
                        in1=brow_sb[:, 0:d].partition_broadcast(rows),
                        op=ALU.add)
                else:
                    nc.scalar.copy(out=q_sb[:rows, irt, b, :], in_=ps[:rows])


def _stage4(nc, tc, cfg, idx_in, kv_full, q_sb, xn_sb, sseg_sb, r8_sb,
            mask_sb, eps_sb, tail=None):
    """Edge stage; when `tail` is given, the output block (LN2 + wo matmul
    + relu + residual + store) for each 128-node block is emitted at that
    block's end so it fills engine gaps in the edge loop (engine streams
    run near in-order; a separate stage5 loop would start only after the
    last edge tile)."""
    d, rt, nt, npt, r = D, cfg.rt, cfg.nt, cfg.npt, cfg.r
    nloc, b_ = cfg.nloc, cfg.b
    inv_sqrt_dh = 1.0 / math.sqrt(DH)
    first_pad_tile = 0 if cfg.mask_all else \
        ((nloc // npt) if nloc % npt else nt)
    cpt = P // 16  # idx columns per tile
    tpb = P // npt  # tiles per 128-node block (= r)
    assert cfg.gt % tpb == 0

    with tc.tile_pool(name="s4g", bufs=2) as gpool, \
         tc.tile_pool(name="s4t", bufs=3) as tpool, \
         tc.tile_pool(name="s5", bufs=3) as spool, \
         tc.tile_pool(name="s4ps", bufs=2, space="PSUM") as pspool, \
         tc.tile_pool(name="s4pb", bufs=2, space="PSUM") as pbpool, \
         tc.tile_pool(name="s4pw", bufs=1, space="PSUM") as pwpool, \
         tc.tile_pool(name="s5py", bufs=1, space="PSUM") as ypool, \
         tc.tile_pool(name="s5pt", bufs=2, space="PSUM") as t5pool:
        for (ts, te) in cfg.groups:
            tg = te - ts
            idx_t = gpool.tile([P, cfg.gt * cpt], I16, tag="idx")
            te_pad = ts + _ceil_div(tg, SUBT) * SUBT
            nc.sync.dma_start(out=idx_t[:, :(te_pad - ts) * cpt],
                              in_=idx_in[:, ts * cpt: te_pad * cpt])
            # dma_gather is limited to 512 indices per instruction
            # (SWDGE descriptor-ring capacity): issue ONE row gather of the
            # whole 2KB kv row (K+V, both batches) per SUBT tiles.
            nsub = _ceil_div(tg, SUBT)
            vr = gpool.tile([P, nsub, SUBT, 4 * d], BF16, tag="vr")
            for j in range(nsub):
                # always a full SUBT-tile gather; idx is host-padded so
                # the tail reads harmless idx-0 rows beyond tg.
                nc.gpsimd.dma_gather(
                    out_ap=vr[:, j, :, :],
                    in_ap=kv_full[:],
                    idxs_ap=idx_t[:, (j * SUBT) * cpt:
                                  (j + 1) * SUBT * cpt],
                    num_idxs=SUBT * P, num_idxs_reg=SUBT * P,
                    elem_size=4 * d, elem_step=4 * d, transpose=False)

            nbv = nbw = None
            for tl in range(tg):
                t_glob = ts + tl
                kvrow = vr[:, tl // SUBT, tl % SUBT, :]
                # q-replication: qrep[slot*16+n, :] = q[node n, :].
                # PE base partitions must be 0/64: contract over a 64-node
                # q block, selecting this tile's 16 nodes via r8[:, tsel].
                po64 = (((t_glob % tpb) * npt) // 64) * 64
                tsel = t_glob % 4
                qps = pspool.tile([P, 2 * d], F32, tag="qps")
                nc.tensor.matmul(
                    out=qps[:], lhsT=r8_sb[po64: po64 + 64, tsel, :],
                    rhs=q_sb[po64: po64 + 64, t_glob // tpb, :, :],
                    start=True, stop=True)
                qrep = tpool.tile([P, 2 * d], BF16, tag="qrep")
                nc.scalar.copy(out=qrep[:], in_=qps[:])
                prod = tpool.tile([P, 2 * d], BF16, tag="prod")
                nc.vector.tensor_tensor(
                    out=prod[:], in0=kvrow[:, 0: 2 * d], in1=qrep[:],
                    op=ALU.mult)
                # reduce 32 -> 16 via a bf16 TT-add (2x mode; tensor_reduce
                # only has 1x uops) then a 16-wide 1x reduce.
                half = tpool.tile([P, 16, 16], BF16, tag="half")
                pv = prod[:].rearrange("p (g u x) -> p g u x", u=2, x=16)
                with nc.allow_low_precision(reason="16-term bf16 partials"):
                    nc.vector.tensor_tensor(
                        out=half[:], in0=pv[:, :, 0, :], in1=pv[:, :, 1, :],
                        op=ALU.add)
                logit = tpool.tile([P, 16], F32, tag="logit")
                nc.vector.tensor_reduce(
                    out=logit[:], in_=half[:],
                    axis=mybir.AxisListType.X, op=ALU.add)
                if tl % tpb == 0:
                    nbv = pbpool.tile([P, 2 * d], F32, tag="nbv")
                    nbw = pwpool.tile([P, 16], F32, tag="nbw")
                wexp = tpool.tile([P, 16], BF16, tag="wexp")
                nc.scalar.activation(out=wexp[:], in_=logit[:],
                                     func=AF.Exp, scale=inv_sqrt_dh)
                if t_glob >= first_pad_tile:
                    nc.vector.tensor_scalar(
                        out=wexp[:], in0=wexp[:],
                        scalar1=mask_sb[:, t_glob:t_glob + 1], scalar2=None,
                        op0=ALU.mult)
                # wv in (b, x, h) order: V columns are x-major so the inner
                # iteration dim (h) is step-1 on all operands -> DVE 2x.
                wv = tpool.tile([P, 2, DH, H], BF16, tag="wv")
                nc.vector.tensor_tensor(
                    out=wv[:],
                    in0=kvrow[:, 2 * d: 4 * d]
                    .rearrange("p (b x h) -> p b x h", b=2, h=H),
                    in1=wexp[:].rearrange("p (b h) -> p b h", b=2)
                    .unsqueeze(2).broadcast_to([P, 2, DH, H]),
                    op=ALU.mult)
                blk_end = (tl % tpb == tpb - 1) or (tl == tg - 1)
                nc.tensor.matmul(
                    out=nbv[:], lhsT=sseg_sb[:, tl % tpb, :],
                    rhs=wv[:].rearrange("p b x h -> p (b x h)"),
                    start=(tl % tpb == 0), stop=blk_end)
                nc.tensor.matmul(
                    out=nbw[:], lhsT=sseg_sb[:, tl % tpb, :], rhs=wexp[:],
                    start=(tl % tpb == 0), stop=blk_end)
                if blk_end:
                    base = npt * (t_glob - (tl % tpb))
                    pr = (tl % tpb) * npt + npt
                    valid = min(P, nloc - base, pr)
                    rec = tpool.tile([P, 16], F32, tag="rec")
                    nc.vector.reciprocal(out=rec[:valid], in_=nbw[:valid])
                    tmp = tpool.tile([P, 2, d], F32, tag="tmp")
                    nc.vector.tensor_tensor(
                        out=tmp[:valid].rearrange("p b (h x) -> p b h x",
                                                  x=DH),
                        in0=nbv[:valid]
                        .rearrange("p (b x h) -> p b h x", b=2, h=H),
                        in1=rec[:valid].rearrange("p (b h) -> p b h", b=2)
                        .unsqueeze(-1).broadcast_to([valid, 2, H, DH]),
                        op=ALU.mult)
                    xn_bt = xn_sb[:valid].rearrange(
                        "p (b t) d -> p t b d", b=2)[:, base // P, :, :]
                    nc.vector.tensor_tensor(
                        out=xn_bt, in0=xn_bt, in1=tmp[:valid], op=ALU.add)
                    if tail is not None:
                        for b in range(b_):
                            _stage5_tile(nc, cfg, xn_sb, eps_sb, tail,
                                         spool, ypool, t5pool,
                                         b, base // P, valid)


def _stage5(nc, tc, cfg, xn_sb, wo_sb, ident_b, eps_sb, gb_sb, brow_sb, out_ext):
    d, rt, nloc = D, cfg.rt, cfg.nloc
    with tc.tile_pool(name="s5", bufs=3) as pool, \
         tc.tile_pool(name="s5p", bufs=2, space="PSUM") as ppool:
        for b in range(cfg.b):
            for irt in range(rt):
                bt = b * rt + irt
                rows = min(P, nloc - irt * P)
                mv = _layer_norm_rs(nc, pool, xn_sb[:rows, bt, :], rows, eps_sb)
                cnb = pool.tile([P, d], BF16, tag="cnb")
                if rows < P:
                    nc.vector.memset(cnb[:], 0.0)
                if cfg.apply_gb2:
                    cn32 = pool.tile([P, d], F32, tag="cn32")
                    nc.vector.tensor_scalar(
                        out=cn32[:rows], in0=xn_sb[:rows, bt, :],
                        scalar1=mv[:rows, 0:1], scalar2=mv[:rows, 1:2],
                        op0=ALU.subtract, op1=ALU.mult)
                    nc.vector.tensor_tensor(
                        out=cn32[:rows], in0=cn32[:rows],
                        in1=gb_sb[:, 2, :].partition_broadcast(rows),
                        op=ALU.mult)
                    nc.vector.tensor_tensor(
                        out=cnb[:rows], in0=cn32[:rows],
                        in1=gb_sb[:, 3, :].partition_broadcast(rows),
                        op=ALU.add)
                else:
                    # normalize on ACT (per-partition scale/bias) to keep
                    # DVE free for the overlapping edge-stage tail
                    b2 = pool.tile([P, 1], F32, tag="b2")
                    nc.vector.tensor_scalar(
                        out=b2[:rows], in0=mv[:rows, 0:1],
                        scalar1=mv[:rows, 1:2], scalar2=-1.0,
                        op0=ALU.mult, op1=ALU.mult)
                    nc.scalar.activation(
                        out=cnb[:rows], in_=xn_sb[:rows, bt, :],
                        func=AF.Identity, bias=b2[:rows],
                        scale=mv[:rows, 1:2])
                y = ppool.tile([P, d], F32, tag="y")
                for ci in range(2):
                    pt = ppool.tile([P, P], BF16, tag="tr5")
                    nc.tensor.transpose(out=pt[:],
                                        in_=cnb[:, ci * P:(ci + 1) * P],
                                        identity=ident_b[:])
                    cnt = pool.tile([P, P], BF16, tag="cnt")
                    nc.scalar.copy(out=cnt[:], in_=pt[:])
                    nc.tensor.matmul(out=y[:], lhsT=cnt[:], rhs=wo_sb[:, ci, :],
                                     start=(ci == 0), stop=(ci == 1))
                if cfg.apply_bo:
                    nc.vector.tensor_tensor(
                        out=y[:rows], in0=y[:rows],
                        in1=brow_sb[:, 3 * d: 4 * d].partition_broadcast(rows),
                        op=ALU.add)
                ysb = pool.tile([P, d], F32, tag="ysb")
                nc.scalar.activation(out=ysb[:rows], in_=y[:rows], func=AF.Relu)
                ot = pool.tile([P, d], F32, tag="ot")
                nc.vector.tensor_tensor(out=ot[:rows], in0=ysb[:rows],
                                        in1=xn_sb[:rows, bt, :], op=ALU.add)
                nc.sync.dma_start(
                    out=out_ext[b * nloc + irt * P:
                                b * nloc + irt * P + rows, :],
                    in_=ot[:rows])


# ------------------------------------------------------------------ host side
def _prep_edges(edges: np.ndarray, cfg: Cfg):
    """Group edges by src, pad each node to cfg.r slots.

    Within each 128-edge tile, positions are SLOT-major (p = slot*npt +
    node_in_tile) so per-node broadcasts have step-1 inner dims on DVE.
    dst is remapped to the 2-phase-AllGather kv_full row layout.

    Returns per-core lists: idx_wrapped [128, nt_pad*8] int16,
    mask [128, nt] f32.
    """
    n, nloc, r, npt, nt = cfg.n, cfg.nloc, cfg.r, cfg.npt, cfg.nt
    src = np.asarray(edges[:, 0], dtype=np.int64)
    dst = np.asarray(edges[:, 1], dtype=np.int64)
    order = np.argsort(src, kind="stable")
    src_s, dst_s = src[order], dst[order]
    counts = np.bincount(src_s, minlength=n)
    assert counts.max() <= r, f"node degree {counts.max()} > r={r}"
    starts = np.zeros(n, dtype=np.int64)
    starts[1:] = np.cumsum(counts)[:-1]
    slot = (np.arange(len(src_s)) - starts[src_s]) + src_s * r
    # kv_full row of global node g after the phased AllGather
    c_dst, j_dst = dst_s // nloc, dst_s % nloc
    bounds = cfg.ag_bounds
    row = np.zeros_like(j_dst)
    row_off = 0
    for lo, hi in zip(bounds[:-1], bounds[1:]):
        in_ph = (j_dst >= lo) & (j_dst < hi)
        row[in_ph] = row_off + c_dst[in_ph] * (hi - lo) + (j_dst[in_ph] - lo)
        row_off += cfg.ncores * (hi - lo)
    dst_pad = np.zeros(n * r, dtype=np.int16)
    mask_pad = np.zeros(n * r, dtype=np.float32)
    dst_pad[slot] = row.astype(np.int16)
    mask_pad[slot] = 1.0
    idx_list, mask_list = [], []
    for c in range(cfg.ncores):
        dp = np.pad(dst_pad[c * nloc * r:(c + 1) * nloc * r],
                    (0, (nt * npt - nloc) * r))
        mp = np.pad(mask_pad[c * nloc * r:(c + 1) * nloc * r],
                    (0, (nt * npt - nloc) * r))
        # node-major -> slot-major within each tile
        dp = dp.reshape(nt, npt, r).transpose(0, 2, 1).reshape(-1)
        mp = mp.reshape(nt, npt, r).transpose(0, 2, 1).reshape(-1)
        dp = np.pad(dp, (0, (cfg.nt_pad - nt) * P))
        # flat position j = T*128 + p, p = slot*npt + node_in_tile
        idx_w = np.tile(dp.reshape(-1, 16).T, (8, 1))  # [128, nt_pad*8]
        mtiles = mp.reshape(nt, P).T.copy()            # [128, nt]
        idx_list.append(np.ascontiguousarray(idx_w, dtype=np.int16))
        mask_list.append(np.ascontiguousarray(mtiles, dtype=np.float32))
    return idx_list, mask_list, counts


def _to_bf16(a):
    import ml_dtypes
    return np.asarray(a, dtype=np.float32).astype(ml_dtypes.bfloat16)


def _seg_mats(cfg: Cfg):
    p = np.arange(P)
    tpb = cfg.r
    # slot-major tiles: edge position p -> node p % npt
    sseg = np.zeros((P, tpb, P), np.float32)
    for k in range(tpb):
        sseg[p, k, cfg.npt * k + p % cfg.npt] = 1.0
    sseg = sseg.reshape(P, tpb * P)
    # r8[n, t, e'] = 1 iff n == t*npt + e' % npt: selects + replicates the
    # 16 q rows of tile-position t within a 64-node q block.
    r8 = np.zeros((64, 4, P), np.float32)
    for t in range(4):
        r8[t * cfg.npt + (p % cfg.npt), t, p] = 1.0
    r8 = np.concatenate([r8, r8], axis=0)  # same content at base 0 and 64
    return sseg, r8.reshape(P, 4 * P)


_PROG_CACHE: dict = {}


def get_program(cfg: Cfg):
    if cfg not in _PROG_CACHE:
        _PROG_CACHE[cfg] = build_program(cfg)
    return _PROG_CACHE[cfg]


def make_cfg(inputs, **overrides) -> Cfg:
    gamma1 = np.asarray(inputs["gamma1"], np.float32)
    beta1 = np.asarray(inputs["beta1"], np.float32)
    gamma2 = np.asarray(inputs["gamma2"], np.float32)
    beta2 = np.asarray(inputs["beta2"], np.float32)
    bqkv = np.concatenate([np.asarray(inputs["bq"], np.float32),
                           np.asarray(inputs["bk"], np.float32),
                           np.asarray(inputs["bv"], np.float32)])
    bo = np.asarray(inputs["bo"], np.float32)
    edges = np.asarray(inputs["edges"])
    n = overrides.get("n", N)
    counts = np.bincount(np.asarray(edges[:, 0], np.int64), minlength=n)
    r = 8
    while r < counts.max():
        r *= 2
    assert r <= P
    return Cfg(
        r=r,
        mask_all=bool(counts.min() < r),
        apply_gb1=not (np.all(gamma1 == 1) and np.all(beta1 == 0)),
        apply_gb2=not (np.all(gamma2 == 1) and np.all(beta2 == 0)),
        apply_bqkv=bool(np.any(bqkv != 0)),
        apply_bo=bool(np.any(bo != 0)),
        **overrides,
    )


def make_in_maps(inputs: dict, cfg: Cfg):
    x = np.asarray(inputs["x"], dtype=np.float32)
    edges = np.asarray(inputs["edges"])
    # V columns permuted x-major (col x*H+h) so the per-edge w broadcast in
    # stage4's wv multiply has a step-1 inner dim (DVE 2x).
    wv_x = np.ascontiguousarray(
        np.asarray(inputs["wv"], np.float32).reshape(D, H, DH)
        .transpose(0, 2, 1).reshape(D, D))
    bv_x = np.ascontiguousarray(
        np.asarray(inputs["bv"], np.float32).reshape(H, DH).T.reshape(D))
    wqkv = np.concatenate([np.asarray(inputs["wq"], np.float32),
                           np.asarray(inputs["wk"], np.float32),
                           wv_x], axis=1)
    wo = np.asarray(inputs["wo"], np.float32)
    bqkv = np.concatenate([np.asarray(inputs["bq"], np.float32),
                           np.asarray(inputs["bk"], np.float32),
                           bv_x])
    bo = np.asarray(inputs["bo"], np.float32)
    gb = np.stack([np.asarray(inputs["gamma1"], np.float32),
                   np.asarray(inputs["beta1"], np.float32),
                   np.asarray(inputs["gamma2"], np.float32),
                   np.asarray(inputs["beta2"], np.float32)])

    idx_list, mask_list, _ = _prep_edges(edges, cfg)
    sseg, r8 = _seg_mats(cfg)
    brow = np.concatenate([bqkv, bo])[None, :].astype(np.float32)

    wqkv_b = _to_bf16(wqkv)
    wo_b = _to_bf16(wo)
    sseg_b = _to_bf16(sseg)
    r8_b = _to_bf16(r8)
    in_maps = []
    for c in range(cfg.ncores):
        lo, hi = c * cfg.nloc, (c + 1) * cfg.nloc
        x_loc = np.ascontiguousarray(x[:, lo:hi, :].reshape(cfg.b * cfg.nloc, D))
        in_maps.append({
            "x": x_loc,
            "wqkv": wqkv_b,
            "wo": wo_b,
            "idx": idx_list[c],
            "mask": mask_list[c],
            "sseg": sseg_b,
            "r8": r8_b,
            "gb": gb,
            "brow": brow,
        })
    return in_maps


def assemble_out(results, cfg: Cfg):
    out = np.empty((cfg.b, cfg.n, D), dtype=np.float32)
    for c in range(cfg.ncores):
        o = np.asarray(results[c]["out"]).reshape(cfg.b, cfg.nloc, D)
        out[:, c * cfg.nloc:(c + 1) * cfg.nloc, :] = o
    return out


LAST_RESULT = None  # BassKernelResults of the most recent kernel() call


def kernel(**inputs) -> np.ndarray:
    global LAST_RESULT
    from concourse.bass_utils import run_bass_kernel_spmd

    cfg = make_cfg(inputs)
    nc = get_program(cfg)
    in_maps = make_in_maps(inputs, cfg)
    LAST_RESULT = run_bass_kernel_spmd(nc, in_maps, list(range(cfg.ncores)))
    return assemble_out(LAST_RESULT.results, cfg)



def _stage5_tile(nc, cfg, xn_sb, eps_sb, tail, pool, ypool, t5pool,
                 b, irt, rows):
    """One output block: LN2 + @wo + relu + residual + store."""
    wo_sb, ident_b, gb_sb, brow_sb, out_ext = tail
    d, rt, nloc = D, cfg.rt, cfg.nloc
    bt = b * rt + irt
    mv = _layer_norm_rs(nc, pool, xn_sb[:rows, bt, :], rows, eps_sb)
    cnb = pool.tile([P, d], BF16, tag="cnb")
    if rows < P:
        nc.vector.memset(cnb[:], 0.0)
    if cfg.apply_gb2:
        cn32 = pool.tile([P, d], F32, tag="cn32")
        nc.vector.tensor_scalar(
            out=cn32[:rows], in0=xn_sb[:rows, bt, :],
            scalar1=mv[:rows, 0:1], scalar2=mv[:rows, 1:2],
            op0=ALU.subtract, op1=ALU.mult)
        nc.vector.tensor_tensor(
            out=cn32[:rows], in0=cn32[:rows],
            in1=gb_sb[:, 2, :].partition_broadcast(rows), op=ALU.mult)
        nc.vector.tensor_tensor(
            out=cnb[:rows], in0=cn32[:rows],
            in1=gb_sb[:, 3, :].partition_broadcast(rows), op=ALU.add)
    else:
        # normalize on ACT (per-partition scale/bias) to spare DVE
        b2 = pool.tile([P, 1], F32, tag="b2")
        nc.vector.tensor_scalar(
            out=b2[:rows], in0=mv[:rows, 0:1], scalar1=mv[:rows, 1:2],
            scalar2=-1.0, op0=ALU.mult, op1=ALU.mult)
        nc.scalar.activation(
            out=cnb[:rows], in_=xn_sb[:rows, bt, :], func=AF.Identity,
            bias=b2[:rows], scale=mv[:rows, 1:2])
    y = ypool.tile([P, d], F32, tag="y")
    for ci in range(2):
        pt = t5pool.tile([P, P], BF16, tag="tr5")
        nc.tensor.transpose(out=pt[:], in_=cnb[:, ci * P:(ci + 1) * P],
                            identity=ident_b[:])
        cnt = pool.tile([P, P], BF16, tag=f"cnt{ci}")
        if ci == 0:
            nc.scalar.copy(out=cnt[:], in_=pt[:])
        else:
            nc.vector.tensor_copy(out=cnt[:], in_=pt[:])
        nc.tensor.matmul(out=y[:], lhsT=cnt[:], rhs=wo_sb[:, ci, :],
                         start=(ci == 0), stop=(ci == 1))
    if cfg.apply_bo:
        nc.vector.tensor_tensor(
            out=y[:rows], in0=y[:rows],
            in1=brow_sb[:, 3 * d: 4 * d].partition_broadcast(rows),
            op=ALU.add)
    ysb = pool.tile([P, d], F32, tag="ysb")
    nc.scalar.activation(out=ysb[:rows], in_=y[:rows], func=AF.Relu)
    ot = pool.tile([P, d], F32, tag="ot")
    nc.vector.tensor_tensor(out=ot[:rows], in0=ysb[:rows],
                            in1=xn_sb[:rows, bt, :], op=ALU.add)
    nc.sync.dma_start(
        out=out_ext[b * nloc + irt * P: b * nloc + irt * P + rows, :],
        in_=ot[:rows])
